# revision 1
# baseline (speedup 1.0000x reference)
"""Trainium2 Bass kernel for nn_AttnModel (gnn_message_passing).

Sharding: data-parallel over graphs B=32 across 8 cores (4 graphs/core).
Only collective: AllReduce of global-layer-norm sum/sumsq of t = nf@W^T+b.

Algebra (per core; x0 = GLN(t) = alpha*(t-m)):
  x_l = a_l*x0 + B_l@V^T      (gated residual stays in span{x0, V-cols})
  x_p_l = relu(P0a_l + B_l@VU)   P0a_l = a_l*(x0@U), via P0a *= om each layer
  zx_l = g0a_l + B_l@vg          g0a_l = a_l*(gw1.x0)
  zh_l = w_l.w2v                 w_l = relu(A_l y_p)
  om = 1-z = sigmoid(-(zpre+gb));  B' = om*(B-w)+w;  P0a *= om; g0a *= om
  out = sigmoid(sum_k x_p_2 * (x_p_2 @ YPY)),  YPY_g = y_p_g^T y_p_g

Layout: "4-stacked" [128, 4096]: partition rows 32g..32g+31 = k (or j-half)
dim of graph g; free = node index within graph. blockdiag lhsT [128,128]
f32r gives a 4-graph-parallel K=32 matmul at 1 cyc/col.

Perf structure: nf is DMA'd naturally (node-major) once per piece and
transposed on-chip by the PE (DMA-transpose of f32 is ~9 GB/s — 40x slower).
The Gram/stat matmuls run first so the AllReduce is issued early and its
latency hides under the transpose + P0/g0 phase. Engine balance: ACT does
exp/sigmoid/relu+psum copies, DVE does adds/muls, Pool does reduce_max,
gating muls and relu copies.
"""

import numpy as np

B_ALL, NPG = 32, 4096
F, C, K, J = 64, 128, 32, 40
EPS = 1e-5
NC_ = 8
GPC = B_ALL // NC_            # 4 graphs per core
NLOC = GPC * NPG              # 16384 nodes per core
Q = NPG                       # 4096
CH = 512
NTOT = float(B_ALL * NPG * C)

_CACHE = {}


def _build(debug=False, collective=True):
    import concourse.bass as bass
    import concourse.mybir as mybir
    import concourse.tile as tile
    import concourse.bacc as bacc

    f32 = mybir.dt.float32
    f32r = mybir.dt.float32r
    bf16 = mybir.dt.bfloat16
    AF = mybir.ActivationFunctionType
    ALU = mybir.AluOpType
    AX = mybir.AxisListType

    nc = bacc.Bacc("TRN2", target_bir_lowering=False, debug=False, num_devices=NC_)

    def din(name, shape):
        return nc.dram_tensor(name, list(shape), f32, kind="ExternalInput")

    nf_d = din("nf", (NLOC, F))
    fragf_d = din("fragf", (B_ALL * J, F))     # full frag (stats), replicated
    fragl_d = din("fragl", (GPC * J, F))       # local 4 graphs' frag rows
    W_d = din("W_in", (C, F))
    b_d = din("b_in", (C, 1))
    U_d = din("U", (C, K))
    V_d = din("V", (C, K))
    q_d = din("q", (K, 1))
    gw1_d = din("gw1", (C, 1))
    gw2_d = din("gw2", (C, 1))
    gb_d = din("gb", (128, 1))
    id_d = din("ident", (128, 128))
    onesrow_d = din("onesrow", (1, 128))
    out_d = nc.dram_tensor("out", [GPC, NPG], f32, kind="ExternalOutput")
    dbg = {}
    def dout(name, shape):
        if debug:
            dbg[name] = nc.dram_tensor("dbg_" + name, list(shape), f32,
                                       kind="ExternalOutput")
        return dbg.get(name)

    NFR = B_ALL * J     # 1280
    NFL = GPC * J       # 160
    NBF = NFR // 128    # 10 row-blocks of 128

    with tile.TileContext(nc) as tc:
        with (
            tc.tile_pool(name="const", bufs=1) as cst,
            tc.tile_pool(name="big", bufs=1) as big,
            tc.tile_pool(name="psb", bufs=2, space="PSUM") as psb,   # 2x[128,1024]
            tc.tile_pool(name="psc", bufs=2, space="PSUM") as psc,   # 2x[128,512]
            tc.tile_pool(name="psg", bufs=1, space="PSUM") as psg,   # Gram
            tc.tile_pool(name="dram", bufs=1, space="DRAM") as drp,
        ):
            def ctile(name, shape, dtype=f32):
                return cst.tile(list(shape), dtype, tag=name, name=name)

            def btile(name, shape, slot, dtype=f32):
                return big.tile(list(shape), dtype, tag=slot, name=name)

            def pchunk(name="pch"):
                return psc.tile([128, CH], f32, tag="pch", name=name)

            def pbig(name="pI"):
                return psb.tile([128, 1024], f32, tag="pI", name=name)

            def ldc(dramt, name, shape):
                t = ctile(name, shape)
                nc.sync.dma_start(t[:], dramt.ap())
                return t

            # ---------------- DMA issue order: big loads interleaved -------
            # half_0 first (gates Gram piece 0), consts, frag, half_1.
            # ---------------- DMA order: ident/W gate the PE pipeline ----
            tId = ldc(id_d, "ident", (128, 128))
            tW = ldc(W_d, "W", (C, F))
            halves = []
            half0 = btile("nfnat_0", (128, Q), "S2")
            for hh in range(2):
                nc.sync.dma_start(
                    half0[:, 2048 * hh:2048 * hh + 2048]
                    .rearrange("p (b f) -> p b f", b=32),
                    nf_d.ap()[4096 * hh:4096 * hh + 4096, :]
                    .rearrange("(b p) f -> p b f", p=128))
            halves.append(half0)

            # ---------------- remaining constants ----------------
            tb = ldc(b_d, "b", (C, 1))
            tU = ldc(U_d, "U", (C, K))
            tV = ldc(V_d, "V", (C, K))
            tq = ldc(q_d, "q", (K, 1))
            tgw1 = ldc(gw1_d, "gw1", (C, 1))
            tgw2 = ldc(gw2_d, "gw2", (C, 1))
            tgb = ldc(gb_d, "gb", (128, 1))
            tOnesRow = ldc(onesrow_d, "onesrow", (1, 128))

            half1 = btile("nfnat_1", (128, Q), "S8")
            for hh in range(2):
                nc.sync.dma_start(
                    half1[:, 2048 * hh:2048 * hh + 2048]
                    .rearrange("p (b f) -> p b f", b=32),
                    nf_d.ap()[NLOC // 2 + 4096 * hh:NLOC // 2 + 4096 * hh + 4096, :]
                    .rearrange("(b p) f -> p b f", p=128))
            halves.append(half1)

            fragN = ctile("fragN", (128, NBF * F))
            nc.sync.dma_start(
                fragN[:].rearrange("p (b f) -> p b f", b=NBF),
                fragf_d.ap().rearrange("(b p) f -> p b f", p=128))
            fragNl = ctile("fragNl", (128, 2 * F))
            nc.sync.dma_start(fragNl[0:128, 0:F], fragl_d.ap()[0:128, :])
            nc.sync.dma_start(fragNl[0:NFL - 128, F:2 * F],
                              fragl_d.ap()[128:NFL, :])

            tOnes128 = ctile("ones128", (128, 1))
            nc.vector.memset(tOnes128[:], 1.0)
            tNgb = ctile("ngb", (128, 1))
            nc.vector.tensor_scalar_mul(tNgb[:], tgb[:], -1.0)
            tNgbH = ctile("ngbh", (128, 1))
            nc.vector.tensor_scalar_mul(tNgbH[:], tgb[:], -0.5)
            tOnes128b = ctile("ones128b", (128, 1), bf16)
            nc.vector.memset(tOnes128b[:], 1.0)
            tZero = ctile("zerof", (128, 128))
            nc.vector.memset(tZero[:], 0.0)

            def zfill(t):
                nc.vector.tensor_copy(t[:], tZero[0:t.shape[0], 0:t.shape[1]])

            tIdbd = ctile("idbd_r", (128, 128), f32r)
            tIdbdH = ctile("idbd_h", (128, 128), f32r)
            tIdbdQ = ctile("idbd_q", (128, 128), f32r)
            zfill(tIdbd)
            zfill(tIdbdH)
            zfill(tIdbdQ)
            for g in range(GPC):
                sl = slice(K * g, K * g + K)
                nc.vector.tensor_copy(tIdbd[sl, sl], tId[0:K, 0:K])
                nc.vector.tensor_scalar_mul(tIdbdH[sl, sl], tId[0:K, 0:K], 0.5)
                nc.vector.tensor_scalar_mul(tIdbdQ[sl, sl], tId[0:K, 0:K], 0.25)
            tSumbd = ctile("sumbd", (128, 128), f32r)
            zfill(tSumbd)
            for g in range(GPC):
                sl = slice(K * g, K * g + K)
                nc.vector.tensor_scalar(tSumbd[sl, sl], tId[0:K, 0:K], 0.0, 1.0,
                                        ALU.mult, ALU.add)   # ones block

            # ---------------- derived weights ----------------
            rhsUg = ctile("rhsUg", (C, K + 2))
            nc.vector.tensor_copy(rhsUg[:, 0:K], tU[:])
            nc.vector.tensor_copy(rhsUg[:, K:K + 1], tgw1[:])
            nc.vector.tensor_copy(rhsUg[:, K + 1:K + 2], tgw2[:])
            pw = pchunk()
            nc.tensor.matmul(pw[0:F, 0:K + 2], tW[:], rhsUg[:], start=True, stop=True)
            tWUg = ctile("WUg", (F, K + 2))
            nc.vector.tensor_copy(tWUg[:], pw[0:F, 0:K + 2])
            pw = pchunk()
            nc.tensor.matmul(pw[0:K, 0:K + 2], tV[:], rhsUg[:], start=True, stop=True)
            tVUg = ctile("VUg", (K, K + 2))
            nc.vector.tensor_copy(tVUg[:], pw[0:K, 0:K + 2])
            pw = pchunk()
            nc.tensor.matmul(pw[0:1, 0:K + 2], tOnes128[:], rhsUg[:], start=True, stop=True)
            tColF = ctile("colF", (1, K + 2))          # [colU | sg1 | sg2] free
            nc.vector.tensor_copy(tColF[:], pw[0:1, 0:K + 2])
            pw = pchunk()
            nc.tensor.transpose(pw[0:K + 2, 0:1], tColF[:], tId[0:1, 0:1])
            tColP = ctile("colP", (K + 2, 1))
            nc.vector.tensor_copy(tColP[:], pw[0:K + 2, 0:1])
            tColUrep = ctile("colUrep", (128, 1))
            for g in range(GPC):
                nc.vector.tensor_copy(tColUrep[K * g:K * g + K, :], tColP[0:K, :])

            # blockdiag lhsT for P0/g0: rows 64*gl..+64 = features of graph
            # 2p+gl; cols K*g..+K = WU (or gw1 replicated) -> out partition K*g
            tM_WU = ctile("M_WU", (128, 128), f32r)
            tM_G1 = ctile("M_G1", (128, 128), f32r)
            zfill(tM_WU)
            zfill(tM_G1)
            for piece in range(2):
                for gl in (0, 1):
                    g = 2 * piece + gl
                    nc.vector.tensor_copy(tM_WU[F * gl:F * gl + F, K * g:K * g + K],
                                          tWUg[:, 0:K])
                    nc.vector.tensor_copy(tM_G1[F * gl:F * gl + F, K * g:K * g + K],
                                          tWUg[:, K:K + 1].broadcast_to([F, K]))

            tBdVUn = ctile("bdVUn", (128, 128), f32r)
            tBdVUp = ctile("bdVUp", (128, 128), f32r)
            tBdVGn = ctile("bdVGn", (128, 128), f32r)
            tBdW2V = ctile("bdW2V", (128, 128), f32r)
            zfill(tBdVUn)
            zfill(tBdVUp)
            zfill(tBdVGn)
            zfill(tBdW2V)
            for g in range(GPC):
                sl = slice(K * g, K * g + K)
                nc.vector.tensor_scalar_mul(tBdVUn[sl, sl], tVUg[:, 0:K], -0.5)
                nc.vector.tensor_copy(tBdVUp[sl, sl], tVUg[:, 0:K])
                nc.vector.tensor_scalar_mul(
                    tBdVGn[sl, sl],
                    tVUg[:, K:K + 1].broadcast_to([K, K]), -0.5)
                nc.vector.tensor_copy(tBdW2V[sl, sl],
                                      tVUg[:, K + 1:K + 2].broadcast_to([K, K]))

            pw = pchunk()
            nc.tensor.transpose(pw[0:F, 0:C], tW[:], tId[:])
            tWT = ctile("WT", (F, C))
            nc.vector.tensor_copy(tWT[:], pw[0:F, 0:C])

            # ------------ Gram stats first (early collective) ------------
            P0a = btile("P0a", (128, Q), "S4", f32r)
            g0a = btile("g0a", (128, Q), "S5", f32r)
            psG = psg.tile([F, F], f32, tag="psG", name="psG")
            psS = psg.tile([F, 1], f32, tag="psS", name="psS")

            halfbf = [btile("halfbf_0", (128, Q), "S9", bf16),
                      btile("halfbf_1", (128, Q), "S6", bf16)]

            def emit_gram(piece):
                hbf = halfbf[piece]
                for hh in range(2):
                    nc.gpsimd.tensor_copy(hbf[:, 2048 * hh:2048 * hh + 2048],
                                          halves[piece][:, 2048 * hh:2048 * hh + 2048])
                for b in range(64):
                    st = (piece == 0 and b == 0)
                    sp = (piece == 1 and b == 63)
                    tile_b = hbf[:, F * b:F * b + F]
                    nc.tensor.matmul(psG[:], tile_b, tile_b,
                                     start=st, stop=sp, skip_group_check=True)
                    nc.tensor.matmul(psS[:], tile_b, tOnes128b[:],
                                     start=st, stop=sp, skip_group_check=True)

            nfTs = [btile("nfT2_0", (128, Q), "S1", f32r),
                    btile("nfT2_1", (128, Q), "S3", f32r)]

            def emit_transposes(piece, cc0, cc1):
                # cc indexes groups of 8 node-blocks -> one [64,1024] copy
                half = halves[piece]
                nfT2 = nfTs[piece]
                for cc in range(cc0, cc1):
                    pf = pbig("pT_pI")
                    gl = cc // 4
                    for t in range(8):
                        b = 8 * cc + t
                        nc.tensor.transpose(
                            pf[0:F, 128 * t:128 * t + 128],
                            half[:, F * b:F * b + F], tId[:])
                    colb = (8 * cc % 32) * 128
                    if cc % 2 == 0:
                        nc.scalar.activation(
                            nfT2[F * gl:F * gl + F, colb:colb + 1024],
                            pf[0:F, 0:1024], AF.Identity)
                    else:
                        nc.vector.tensor_copy(
                            nfT2[F * gl:F * gl + F, colb:colb + 1024],
                            pf[0:F, 0:1024])

            def emit_p0_chunk(piece, j2):
                nfT2 = nfTs[piece]
                rsl = slice(64 * piece, 64 * piece + 64)
                pP = pbig("pP0")
                pG0 = pbig("pG0")
                for s in range(2):
                    cols = slice(1024 * j2 + CH * s, 1024 * j2 + CH * s + CH)
                    nc.tensor.matmul(pP[:, CH * s:CH * s + CH], tM_WU[:],
                                     nfT2[:, cols], start=True, stop=True)
                    nc.tensor.matmul(pG0[:, CH * s:CH * s + CH], tM_G1[:],
                                     nfT2[:, cols], start=True, stop=True)
                cols2 = slice(1024 * j2, 1024 * j2 + 1024)
                nc.scalar.activation(P0a[rsl, cols2], pP[rsl, :], AF.Identity)
                nc.vector.tensor_copy(g0a[rsl, cols2], pG0[rsl, :])

            emit_gram(0)
            emit_transposes(0, 0, 4)      # fills PE while half_1 DMA lands
            emit_gram(1)
            emit_transposes(0, 4, 8)

            # ---- local stats -> AllReduce (issued early; latency hides
            # under the transpose/P0 phase below) ----
            tGs = ctile("Gs", (F, F + 1))
            nc.vector.tensor_copy(tGs[:, 0:F], psG[:])
            nc.vector.tensor_copy(tGs[:, F:F + 1], psS[:])
            pf = pchunk()
            nc.tensor.matmul(pf[0:F, 0:C], tGs[:, 0:F], tWT[:], start=True, stop=True)
            tGW = ctile("GW", (F, C))
            nc.vector.tensor_mul(tGW[:], pf[0:F, 0:C], tWT[:])
            pf = pchunk()
            nc.tensor.matmul(pf[0:C, 0:1], tGW[:], tOnes128[0:F, :],
                             start=True, stop=True)              # quad_c
            nc.tensor.matmul(pf[0:C, 1:2], tWT[:], tGs[:, F:F + 1],
                             start=True, stop=True)              # ws_c
            tM5 = ctile("M5", (128, 5))
            nc.vector.tensor_copy(tM5[:, 0:2], pf[0:C, 0:2])
            nc.vector.tensor_copy(tM5[:, 2:3], tb[:])
            nc.vector.tensor_mul(tM5[:, 3:4], tb[:], tb[:])
            nc.vector.tensor_mul(tM5[:, 4:5], tb[:], tM5[:, 1:2])
            pf = pchunk()
            nc.tensor.matmul(pf[0:5, 0:1], tM5[:], tOnes128[:], start=True, stop=True)
            st5 = ctile("st5", (5, 1))
            nc.vector.tensor_copy(st5[:], pf[0:5, 0:1])
            pf2 = pchunk()
            nc.tensor.transpose(pf2[0:1, 0:5], st5[:], tId[0:5, 0:5])
            tST = ctile("stat", (1, 12))
            nc.vector.tensor_copy(tST[:, 0:5], pf2[0:1, 0:5])
            # [0]=quad [1]=ws [2]=b [3]=b2 [4]=bws
            nc.vector.tensor_scalar(tST[:, 5:6], tST[:, 2:3], float(NLOC), None,
                                    ALU.mult)
            nc.vector.tensor_add(tST[:, 5:6], tST[:, 5:6], tST[:, 1:2])
            nc.vector.tensor_scalar(tST[:, 6:7], tST[:, 4:5], 2.0, None, ALU.mult)
            nc.vector.tensor_add(tST[:, 6:7], tST[:, 6:7], tST[:, 0:1])
            nc.vector.tensor_scalar(tST[:, 7:8], tST[:, 3:4], float(NLOC), None,
                                    ALU.mult)
            nc.vector.tensor_add(tST[:, 6:7], tST[:, 6:7], tST[:, 7:8])

            cin = ctile("cin", (1, 128))
            nc.vector.memset(cin[:], 0.0)
            nc.vector.tensor_copy(cin[:, 0:1], tST[:, 5:6])
            nc.vector.tensor_copy(cin[:, 1:2], tST[:, 6:7])
            db_in = drp.tile([1, 128], f32, name="db_in")
            db_out = drp.tile([1, 128], f32, name="db_out")
            nc.sync.dma_start(db_in[:], cin[:])
            if collective:
                nc.gpsimd.collective_compute(
                    "AllReduce", mybir.AluOpType.add,
                    replica_groups=[list(range(NC_))],
                    ins=[db_in.opt()], outs=[db_out.opt()],
                )
            else:
                nc.sync.dma_start(db_out[:], db_in[:])
            cout = ctile("cout", (1, 128))
            nc.sync.dma_start(cout[:], db_out[:])

            # ---- transposes + P0raw/g0raw while the collective flies ----


            # ---------------- frag path (also in collective shadow) -------
            fragT = ctile("fragT", (F, NFR))
            for c0 in range(0, NBF, 4):
                nb = min(4, NBF - c0)
                pf = pchunk()
                for t in range(nb):
                    b = c0 + t
                    nc.tensor.transpose(pf[0:F, 128 * t:128 * t + 128],
                                        fragN[:, F * b:F * b + F], tId[:])
                nc.scalar.activation(fragT[:, 128 * c0:128 * (c0 + nb)],
                                     pf[0:F, 0:128 * nb], AF.Identity)
            ysT = ctile("ysT", (C, NFR))
            for c0 in range(0, NFR, CH):
                w_ = min(CH, NFR - c0)
                pf = pchunk()
                nc.tensor.matmul(pf[:, 0:w_], tWT[:], fragT[:, c0:c0 + w_],
                                 start=True, stop=True)
                nc.scalar.activation(ysT[:, c0:c0 + w_], pf[:, 0:w_],
                                     AF.Identity, bias=tb[:], scale=1.0)
            for _j2 in range(4):
                emit_p0_chunk(0, _j2)

            fsums = ctile("fsums", (128, 5))
            nc.vector.reduce_sum(fsums[:, 0:1], ysT[:], axis=AX.X)
            for ci, c0 in enumerate(range(0, NFR, CH)):
                w_ = min(CH, NFR - c0)
                pf = pchunk()
                nc.scalar.activation(pf[:, 0:w_], ysT[:, c0:c0 + w_], AF.Square,
                                     accum_out=fsums[:, 2 + ci:3 + ci])
            nc.vector.tensor_add(fsums[:, 1:2], fsums[:, 2:3], fsums[:, 3:4])
            nc.vector.tensor_add(fsums[:, 1:2], fsums[:, 1:2], fsums[:, 4:5])
            pf = pchunk()
            nc.tensor.matmul(pf[0:2, 0:1], fsums[:, 0:2], tOnes128[:],
                             start=True, stop=True)
            fs2 = ctile("fs2", (2, 1))
            nc.vector.tensor_copy(fs2[:], pf[0:2, 0:1])
            pf2 = pchunk()
            nc.tensor.transpose(pf2[0:1, 0:2], fs2[:], tId[0:2, 0:2])
            tFS = ctile("fragstat", (1, 8))
            nc.vector.tensor_copy(tFS[:, 0:2], pf2[0:1, 0:2])
            nfr = float(C * NFR)
            nc.vector.tensor_scalar_mul(tFS[:, 2:4], tFS[:, 0:2], 1.0 / nfr)
            nc.vector.tensor_mul(tFS[:, 4:5], tFS[:, 2:3], tFS[:, 2:3])
            nc.vector.tensor_sub(tFS[:, 5:6], tFS[:, 3:4], tFS[:, 4:5])
            nc.vector.tensor_scalar_add(tFS[:, 5:6], tFS[:, 5:6], EPS)
            nc.scalar.activation(tFS[:, 6:7], tFS[:, 5:6], AF.Sqrt)
            nc.vector.reciprocal(tFS[:, 7:8], tFS[:, 6:7])                  # a2
            nc.vector.tensor_mul(tFS[:, 4:5], tFS[:, 7:8], tFS[:, 2:3])
            nc.vector.tensor_scalar_mul(tFS[:, 4:5], tFS[:, 4:5], -1.0)    # -a2*m2
            tA2c = ctile("a2c", (128, 2))
            pf = pchunk()
            nc.tensor.matmul(pf[0:128, 0:1], tOnesRow[:], tFS[:, 7:8],
                             start=True, stop=True)
            nc.tensor.matmul(pf[0:128, 1:2], tOnesRow[:], tFS[:, 4:5],
                             start=True, stop=True)
            nc.vector.tensor_copy(tA2c[:], pf[0:128, 0:2])

            # local frag -> normalized ys (f32r) -> y_p smalls
            fragTl = ctile("fragTl", (F, NFL))
            pf = pchunk()
            nc.tensor.transpose(pf[0:F, 0:128], fragNl[:, 0:F], tId[:])
            nc.tensor.transpose(pf[0:F, 128:NFL], fragNl[0:NFL - 128, F:2 * F],
                                tId[0:NFL - 128, 0:NFL - 128])
            nc.vector.tensor_copy(fragTl[:], pf[0:F, 0:NFL])
            ysTl = ctile("ysTl", (C, NFL))
            pf = pchunk()
            nc.tensor.matmul(pf[:, 0:NFL], tWT[:], fragTl[:], start=True, stop=True)
            nc.scalar.activation(ysTl[:], pf[:, 0:NFL], AF.Identity,
                                 bias=tb[:], scale=1.0)
            ysnl = ctile("ysnl", (C, NFL), f32r)
            nc.scalar.activation(ysnl[:], ysTl[:], AF.Identity,
                                 bias=tA2c[:, 1:2], scale=tA2c[:, 0:1])
            tVr = ctile("Vr", (C, K), f32r)
            nc.vector.tensor_copy(tVr[:], tV[:])
            ypT = ctile("ypT", (K, NFL))
            for g in range(GPC):
                pf = pchunk()
                nc.tensor.matmul(pf[0:K, 0:J], tVr[:], ysnl[:, J * g:J * g + J],
                                 start=True, stop=True)
                nc.scalar.activation(ypT[:, J * g:J * g + J], pf[0:K, 0:J],
                                     AF.Relu, scale=tq[:])
            emit_transposes(1, 0, 8)

            tBdYT0 = ctile("bdYT0", (128, 128), f32r)
            tBdYT1 = ctile("bdYT1", (128, 128), f32r)
            zfill(tBdYT0)
            zfill(tBdYT1)
            for g in range(GPC):
                sl = slice(K * g, K * g + K)
                nc.vector.tensor_copy(tBdYT0[sl, sl], ypT[:, J * g:J * g + K])
                nc.vector.tensor_copy(tBdYT1[sl, K * g:K * g + (J - K)],
                                      ypT[:, J * g + K:J * g + J])
            ynat = ctile("ynat", (2 * K, 128))
            nc.vector.memset(ynat[:], 0.0)
            for g in range(GPC):
                pf = pchunk()
                nc.tensor.transpose(pf[0:J, 0:K], ypT[:, J * g:J * g + J],
                                    tId[0:K, 0:K])
                nc.vector.tensor_copy(ynat[0:J, K * g:K * g + K], pf[0:J, 0:K])
            tBdYPY = ctile("bdYPY", (128, 128), f32r)
            zfill(tBdYPY)
            for g in range(GPC):
                pf = pchunk()
                nc.tensor.matmul(pf[0:K, 0:K], ynat[:, K * g:K * g + K],
                                 ynat[:, K * g:K * g + K], start=True, stop=True)
                nc.vector.tensor_copy(tBdYPY[K * g:K * g + K, K * g:K * g + K],
                                      pf[0:K, 0:K])

            # ---- alpha/bias from the AllReduce result; normalize ----
            tGS = ctile("gstat", (1, 8))
            nc.vector.tensor_scalar_mul(tGS[:, 0:2], cout[:, 0:2], 1.0 / NTOT)
            nc.vector.tensor_mul(tGS[:, 2:3], tGS[:, 0:1], tGS[:, 0:1])
            nc.vector.tensor_sub(tGS[:, 2:3], tGS[:, 1:2], tGS[:, 2:3])
            nc.vector.tensor_scalar_add(tGS[:, 2:3], tGS[:, 2:3], EPS)
            nc.scalar.activation(tGS[:, 3:4], tGS[:, 2:3], AF.Sqrt)
            nc.vector.reciprocal(tGS[:, 4:5], tGS[:, 3:4])              # alpha
            nc.vector.tensor_mul(tGS[:, 5:6], tGS[:, 4:5], tGS[:, 0:1])
            nc.vector.tensor_scalar_mul(tGS[:, 5:6], tGS[:, 5:6], -1.0)  # -am
            nc.vector.tensor_mul(tGS[:, 6:7], tGS[:, 5:6], tColF[:, K:K + 1])
            tAB = ctile("alphab", (128, 3))
            pf = pchunk()
            for ii, cidx in [(0, 4), (1, 5), (2, 6)]:
                nc.tensor.matmul(pf[0:128, ii:ii + 1], tOnesRow[:],
                                 tGS[:, cidx:cidx + 1], start=True, stop=True)
            nc.vector.tensor_copy(tAB[:], pf[0:128, 0:3])
            tBiasP0 = ctile("biasP0", (128, 1))
            nc.vector.tensor_mul(tBiasP0[:], tColUrep[:], tAB[:, 1:2])
            # swap the ACT table to the exp/tanh set now (after the last
            # sqrt) so the E-phase does not pay the 1.3us load
            tWarm = ctile("warm", (1, 1))
            nc.scalar.activation(tWarm[:], tGS[:, 0:1], AF.Exp)
            # xp = relu(alpha*P0raw + biasP0) fused on ACT; P0a/g0a
            # normalized in place on Pool in parallel
            # piece-1 P0 chunks interleaved with fused xp relu so
            # layer 0 is unblocked per column chunk, not at the end
            xp = btile("xp", (128, Q), "S6", f32r)
            for _j2 in range(4):
                emit_p0_chunk(1, _j2)
                xc = slice(1024 * _j2, 1024 * _j2 + 1024)
                nc.scalar.activation(xp[:, xc], P0a[:, xc], AF.Relu,
                                     bias=tBiasP0[:], scale=tAB[:, 0:1])
            nc.gpsimd.tensor_scalar(P0a[:], P0a[:], tAB[:, 0:1], tBiasP0[:],
                                    ALU.mult, ALU.add)
            nc.gpsimd.tensor_scalar(g0a[:], g0a[:], tAB[:, 0:1], tAB[:, 2:3],
                                    ALU.mult, ALU.add)
            if debug:
                d = dout("P0a", (128, Q)); nc.sync.dma_start(d.ap(), P0a[:])
                d = dout("g0a", (128, Q)); nc.sync.dma_start(d.ap(), g0a[:])
                d = dout("gstat", (1, 7)); nc.sync.dma_start(d.ap(), tGS[:, 0:7])
                d = dout("xp0", (128, Q)); nc.sync.dma_start(d.ap(), xp[:].bitcast(f32))

            # ---------------- layers 0,1 ----------------
            Bst = None
            for l in range(2):
                bdYT = [tBdYT0, tBdYT1]
                # single-pass online softmax: exp each 1024-chunk with its
                # chunk-local max n_c; the rescale exp(max_c-max_g)/S_g folds
                # into per-chunk bdYtil stationaries.
                nmx = ctile("nmx%d" % l, (128, 12))   # n_c = -max_c
                E = btile("E%d" % l, (128, 2 * Q), "S1", f32r)
                S = ctile("S%d" % l, (128, 16))
                fsc = ctile("fsc%d" % l, (128, 12))
                if l == 0:
                    bdY0c = [ctile("bdY0c_%d" % c, (128, 128), f32r)
                             for c in range(4)]
                    bdY1c = [ctile("bdY1c_%d" % c, (128, 128), f32r)
                             for c in range(4)]
                    for c in range(4):
                        zfill(bdY0c[c])
                        zfill(bdY1c[c])
                    bdYc = (bdY0c, bdY1c)
                else:
                    bdY0c, bdY1c = bdYc
                for h in range(2):
                    for cq in range(4):
                        pI = pbig("pI")
                        for cc in range(2):
                            cs = slice(1024 * cq + CH * cc,
                                       1024 * cq + CH * cc + CH)
                            nc.tensor.matmul(pI[:, CH * cc:CH * cc + CH],
                                             bdYT[h][:], xp[:, cs],
                                             start=True, stop=True)
                        nc.vector.reduce_max(nmx[:, 4 * h + cq:4 * h + cq + 1],
                                             pI[:], axis=AX.X, negate=True)
                        nc.scalar.activation(
                            E[:, Q * h + 1024 * cq:Q * h + 1024 * cq + 1024],
                            pI[:], AF.Exp,
                            bias=nmx[:, 4 * h + cq:4 * h + cq + 1],
                            accum_out=S[:, 4 * h + cq:4 * h + cq + 1])
                    # combine + rescale for this half immediately (does not
                    # wait the other half's reduces in the engine queues)
                    c0 = 4 * h
                    nc.vector.tensor_reduce(nmx[:, 8 + h:9 + h],
                                            nmx[:, c0:c0 + 4], AX.X, ALU.min)
                    nc.vector.tensor_scalar(fsc[:, c0:c0 + 4],
                                            nmx[:, c0:c0 + 4],
                                            nmx[:, 8 + h:9 + h], None,
                                            ALU.subtract)
                    nc.scalar.activation(fsc[:, c0:c0 + 4], fsc[:, c0:c0 + 4],
                                         AF.Exp, scale=-1.0)
                    nc.vector.tensor_mul(S[:, c0:c0 + 4], S[:, c0:c0 + 4],
                                         fsc[:, c0:c0 + 4])
                    nc.vector.tensor_reduce(S[:, 12 + h:13 + h],
                                            S[:, c0:c0 + 4], AX.X, ALU.add)
                    nc.vector.reciprocal(S[:, 14 + h:15 + h],
                                         S[:, 12 + h:13 + h])
                    nc.vector.tensor_scalar(fsc[:, c0:c0 + 4],
                                            fsc[:, c0:c0 + 4],
                                            S[:, 14 + h:15 + h], None,
                                            ALU.mult)
                    bdYh = bdY0c if h == 0 else bdY1c
                    ysrc = ynat[0:K, :] if h == 0 else ynat[K:2 * K, :]
                    for c in range(4):
                        for g in range(GPC):
                            sl = slice(K * g, K * g + K)
                            nc.gpsimd.tensor_scalar(bdYh[c][sl, sl],
                                                    ysrc[:, sl],
                                                    fsc[sl, c0 + c:c0 + c + 1],
                                                    None, ALU.mult)
                # per-chunk: w = relu(A y_p); zs seeded in psum; om via
                # tanh (same ACT table set as exp); negated-B recursion:
                # nB stores -B; VG/VU stationaries pre-negated.
                wt = btile("w%d" % l, (128, Q), "S7", f32r)
                om = btile("om%d" % l, (128, Q), "S8")
                md = (btile("md%d" % l, (128, Q), "S9", f32r) if l == 1
                      else None)
                Bn = (btile("B%d" % l, (128, Q), "S3", f32r) if l == 0
                      else None)
                for cc in range(8):
                    cols = slice(CH * cc, CH * cc + CH)
                    pWZ = pbig("pWZ")
                    pW = pWZ[:, 0:CH]
                    pZ = pWZ[:, CH:2 * CH]
                    nc.tensor.matmul(pW, bdY0c[cc // 2][:], E[:, cols],
                                     start=True, stop=False)
                    nc.tensor.matmul(pW, bdY1c[cc // 2][:],
                                     E[:, Q + CH * cc:Q + CH * cc + CH],
                                     start=False, stop=True)
                    nc.scalar.activation(wt[:, cols], pW, AF.Relu)
                    if Bst is not None:
                        # md = nB1 + w = 0.5*Bst + w (Bst stores 2*nB1)
                        nc.vector.scalar_tensor_tensor(
                            md[:, cols], Bst[:, cols], 0.5, wt[:, cols],
                            ALU.mult, ALU.add)
                    nc.tensor.matmul(pZ, tBdW2V[:], wt[:, cols],
                                     start=True, stop=False)
                    if Bst is not None:
                        nc.tensor.matmul(pZ, tBdVGn[:], Bst[:, cols],
                                         start=False, stop=False)
                    nc.tensor.matmul(pZ, tIdbd if l == 0 else tIdbdH,
                                     g0a[:, cols], start=False, stop=True)
                    # t = tanh(-(zs+gb)/2); om = 0.5+0.5t never materialized:
                    # consumers fuse (t+-1), stationaries absorb the 2x
                    nc.scalar.activation(om[:, cols], pZ, AF.Tanh,
                                         bias=tNgbH[:], scale=-0.5)
                    if Bst is None:
                        # 2*nB1 = (t - 1)*w
                        nc.vector.scalar_tensor_tensor(
                            Bn[:, cols], om[:, cols], 1.0, wt[:, cols],
                            ALU.subtract, ALU.mult)
                    else:
                        # 2*(md*om) = (t + 1)*md ; nB2 never materialized
                        nc.vector.scalar_tensor_tensor(
                            md[:, cols], om[:, cols], 1.0, md[:, cols],
                            ALU.add, ALU.mult)
                    # P0a doubles each layer: (t + 1)*P0a = 2*P0a*om
                    nc.vector.scalar_tensor_tensor(
                        P0a[:, cols], om[:, cols], 1.0, P0a[:, cols],
                        ALU.add, ALU.mult)
                    if l == 0:
                        nc.vector.scalar_tensor_tensor(
                            g0a[:, cols], om[:, cols], 1.0, g0a[:, cols],
                            ALU.add, ALU.mult)
                if Bn is not None:
                    Bst = Bn
                if debug:
                    d = dout("w%d" % l, (128, Q)); nc.sync.dma_start(d.ap(), wt[:].bitcast(f32))
                    d = dout("om%d" % l, (128, Q)); nc.sync.dma_start(d.ap(), om[:])
                # x_p for layer l+1 = relu(P0a + B@VU), per chunk
                xp = btile("xp_%d" % (l + 1), (128, Q), "S6", f32r)
                for cc in range(8):
                    cols = slice(CH * cc, CH * cc + CH)
                    pX = pchunk("pX")
                    if l == 0:
                        # VU@B1 = (-VU/2)@(2*nB1)
                        nc.tensor.matmul(pX[:], tBdVUn[:], Bst[:, cols],
                                         start=True, stop=False)
                    else:
                        # VU@B2 = (-VU/2)@(2*md*om) + VU@w
                        nc.tensor.matmul(pX[:], tBdVUn[:], md[:, cols],
                                         start=True, stop=False)
                        nc.tensor.matmul(pX[:], tBdVUp[:], wt[:, cols],
                                         start=False, stop=False)
                    nc.tensor.matmul(pX[:], tIdbdH if l == 0 else tIdbdQ,
                                     P0a[:, cols], start=False, stop=True)
                    nc.scalar.activation(xp[:, cols], pX[:], AF.Relu)
                if debug:
                    d = dout("xp%d" % (l + 1), (128, Q))
                    nc.sync.dma_start(d.ap(), xp[:].bitcast(f32))

            # ---------------- final ----------------
            tmp = btile("tmp", (128, Q), "S5", f32r)
            sOut = btile("sOut", (128, Q), "S2")
            for j2 in range(4):
                pP = pbig("pF")
                for s in range(2):
                    cols = slice(1024 * j2 + CH * s, 1024 * j2 + CH * s + CH)
                    nc.tensor.matmul(pP[:, CH * s:CH * s + CH], tBdYPY[:],
                                     xp[:, cols], start=True, stop=True)
                cols2 = slice(1024 * j2, 1024 * j2 + 1024)
                nc.vector.tensor_mul(tmp[:, cols2], pP[:], xp[:, cols2])
                pS = pbig("pS")
                for s in range(2):
                    cols = slice(1024 * j2 + CH * s, 1024 * j2 + CH * s + CH)
                    nc.tensor.matmul(pS[:, CH * s:CH * s + CH], tSumbd[:],
                                     tmp[:, cols], start=True, stop=True)
                nc.scalar.activation(sOut[:, cols2], pS[:], AF.Tanh, scale=0.5)
                nc.vector.tensor_scalar(sOut[:, cols2], sOut[:, cols2], 0.5, 0.5,
                                        ALU.mult, ALU.add)
            for g in range(GPC):
                nc.sync.dma_start(out_d.ap()[g:g + 1, :],
                                  sOut[K * g:K * g + 1, :])


    nc.compile()
    return nc


def _get_program(debug=False):
    key = "nc_dbg" if debug else "nc"
    if key not in _CACHE:
        _CACHE[key] = _build(debug)
    return _CACHE[key]


def make_in_maps(inputs):
    nf = np.ascontiguousarray(np.asarray(inputs["node_feats"], np.float32))
    frag = np.ascontiguousarray(
        np.asarray(inputs["frag_emb"], np.float32).reshape(B_ALL * J, F))
    W = np.ascontiguousarray(np.asarray(inputs["W_in"], np.float32))
    b = np.asarray(inputs["b_in"], np.float32).reshape(C, 1)
    U = np.ascontiguousarray(np.asarray(inputs["U"], np.float32))
    V = np.ascontiguousarray(np.asarray(inputs["V"], np.float32))
    q = np.asarray(inputs["q"], np.float32).reshape(K, 1)
    gW = np.asarray(inputs["gate_W"], np.float32).reshape(2 * C)
    gb = np.asarray(inputs["gate_b"], np.float32).reshape(1)
    in_maps = []
    for c in range(NC_):
        in_maps.append({
            "nf": nf[c * NLOC:(c + 1) * NLOC],
            "fragf": frag,
            "fragl": np.ascontiguousarray(frag[c * GPC * J:(c + 1) * GPC * J]),
            "W_in": W, "b_in": b, "U": U, "V": V, "q": q,
            "gw1": np.ascontiguousarray(gW[:C].reshape(C, 1)),
            "gw2": np.ascontiguousarray(gW[C:].reshape(C, 1)),
            "gb": np.full((128, 1), gb[0], np.float32),
            "ident": np.eye(128, dtype=np.float32),
            "onesrow": np.ones((1, 128), np.float32),
        })
    return in_maps


def kernel(**inputs):
    from concourse.bass_utils import run_bass_kernel_spmd

    nc = _get_program()
    in_maps = make_in_maps(inputs)
    res = run_bass_kernel_spmd(nc, in_maps, core_ids=list(range(NC_)))
    return np.concatenate([r["out"] for r in res.results], axis=0)

